# revision 10
# baseline (speedup 1.0000x reference)
"""Trainium2 Bass kernel for nn_EncoderLayer (B=2, L=2048, D=1024, 16 heads, FFN 4096).

Strategy: sequence-parallel over the 8 cores (core c owns batch c//4, query rows
(c%4)*512 .. +512).  Each core recomputes the full K projection for its batch
(4.3 GF duplicated work) which avoids all collectives; everything else is local.

Device layout: activations are kept transposed (features on partitions, tokens on
the free dim) so every matmul contracts over the partition dim.  The reference
interprets the projection output as [l, HD, N] (feature = d*16 + h), so the host
permutes wq/wk columns and wo rows to make heads contiguous 64-blocks; outputs
come back in natural feature order.

Numerics: matmuls in bf16 with fp32 PSUM accumulation; softmax, layernorm, gelu
in fp32 on ACT/DVE.  Softmax normalization uses an appended ones-column in the
K operand so the row-sum falls out of the same matmul that computes attn @ K.

v2: kaug derived from kT via PE transposes (instead of a second matmul pass);
softmax denominators batched into one reciprocal + one broadcast multiply;
wo/xq prefetched during attention; w2 fully prefetched during FFN; LN uses
ACT Square + Rsqrt; 1/sqrt(HD) folded into wq on the host; chunked output DMA.
"""

import numpy as np
import ml_dtypes

B, L, D, NH, HD, FF = 2, 2048, 1024, 16, 64, 4096
LQ = 512  # query rows per core
NCORES = 8
EPS = 1e-5
DC = D // 128  # 8 feature chunks
MC = L // 128  # 16 key chunks
FC = FF // 128  # 32 ffn chunks
BF16NP = ml_dtypes.bfloat16

_cache = {}
LAST_RESULTS = None


def _build_nc():
    import concourse.bass as bass
    import concourse.tile as tile
    from concourse import bacc, mybir
    from contextlib import ExitStack

    f32 = mybir.dt.float32
    bf16 = mybir.dt.bfloat16
    AF = mybir.ActivationFunctionType
    OP = mybir.AluOpType

    nc = bacc.Bacc("TRN2", debug=False, target_bir_lowering=False)

    # ---- DRAM I/O ----
    xb_d = nc.dram_tensor("xb", [D, L], bf16, kind="ExternalInput").ap()
    xqb_d = nc.dram_tensor("xqb", [D, LQ], bf16, kind="ExternalInput").ap()
    xq_d = nc.dram_tensor("xq", [D, LQ], f32, kind="ExternalInput").ap()
    wq_d = nc.dram_tensor("wq", [D, D], bf16, kind="ExternalInput").ap()
    wk_d = nc.dram_tensor("wk", [D, D], bf16, kind="ExternalInput").ap()
    wo_d = nc.dram_tensor("wo", [D, D], bf16, kind="ExternalInput").ap()
    w1_d = nc.dram_tensor("w1", [FC, 128, D], bf16, kind="ExternalInput").ap()
    w2_d = nc.dram_tensor("w2", [FC, 128, D], bf16, kind="ExternalInput").ap()
    ident_d = nc.dram_tensor("ident", [128, 128], bf16, kind="ExternalInput").ap()
    bb1_d = nc.dram_tensor("bb1", [FF], f32, kind="ExternalInput").ap()
    bb2_d = nc.dram_tensor("bb2", [D], f32, kind="ExternalInput").ap()
    g1_d = nc.dram_tensor("g1", [D], f32, kind="ExternalInput").ap()
    b1_d = nc.dram_tensor("b1", [D], f32, kind="ExternalInput").ap()
    g2_d = nc.dram_tensor("g2", [D], f32, kind="ExternalInput").ap()
    b2_d = nc.dram_tensor("b2", [D], f32, kind="ExternalInput").ap()
    out_d = nc.dram_tensor("out", [D, LQ], f32, kind="ExternalOutput").ap()

    xb_v = xb_d.rearrange("(c p) m -> p c m", p=128)
    xqb_v = xqb_d.rearrange("(c p) l -> p c l", p=128)
    xq_v = xq_d.rearrange("(c p) l -> p c l", p=128)
    wq_v = wq_d.rearrange("(c p) f -> p c f", p=128)
    wk_v = wk_d.rearrange("(c p) f -> p c f", p=128)
    wo_v = wo_d.rearrange("(c p) f -> p c f", p=128)
    bb1_v = bb1_d.rearrange("(c p) -> p c", p=128)
    bb2_v = bb2_d.rearrange("(c p) -> p c", p=128)
    g1_v = g1_d.rearrange("(c p) -> p c", p=128)
    b1_v = b1_d.rearrange("(c p) -> p c", p=128)
    g2_v = g2_d.rearrange("(c p) -> p c", p=128)
    b2_v = b2_d.rearrange("(c p) -> p c", p=128)
    out_v = out_d.rearrange("(c p) l -> p c l", p=128)

    with tile.TileContext(nc, pool_alloc_mode="queue") as tc, ExitStack() as top:
        consts = top.enter_context(tc.tile_pool(name="consts", bufs=1))
        dramsc = top.enter_context(tc.tile_pool(name="dramsc", bufs=2, space="DRAM"))

        def bcast(dst, src, tag):
            """Broadcast src [1, N] to dst [P, N] via a DRAM round-trip
            (0-stride partition DMA; gpsimd partition_broadcast fails codegen)."""
            sc = dramsc.tile([1, src.shape[-1]], src.dtype, tag=tag, name=f"sc_{tag}")
            nc.sync.dma_start(sc, src)
            nc.sync.dma_start(dst, sc[0].partition_broadcast(dst.shape[0]))

        sm = top.enter_context(tc.tile_pool(name="smalls", bufs=1))
        sm2 = top.enter_context(tc.tile_pool(name="smalls2", bufs=2))

        with tc.tile_pool(name="mid", bufs=1) as mid:
            hT = mid.tile([128, DC, LQ], f32, tag="hT")
            hb = mid.tile([128, DC, LQ], bf16, tag="hb")

            with tc.tile_pool(name="kq", bufs=1) as kq:
                kT = kq.tile([128, DC, L], bf16, tag="kT")
                kaug = kq.tile([128, MC, NH * 65], bf16, tag="kaug")
                qT = kq.tile([128, DC, LQ], bf16, tag="qT")
                r1T = kq.tile([128, DC, LQ], bf16, tag="r1T")
                kaug_h = kaug.rearrange("p m (h e) -> p m h e", e=65)

                # ---- Phase 1: projections ----
                with tc.tile_pool(name="p1", bufs=1) as p1, \
                     tc.tile_pool(name="p1w", bufs=1) as p1w, \
                     tc.tile_pool(name="psA", bufs=4, space="PSUM") as psA:
                    # q-path inputs first so the tensor engine starts early
                    xqb = p1.tile([128, DC, LQ], bf16, tag="xqb")
                    nc.sync.dma_start(xqb, xqb_v)
                    wq_sb = p1w.tile([128, DC, D], bf16, tag="wproj")
                    nc.sync.dma_start(wq_sb, wq_v)
                    wk_sb = p1w.tile([128, DC, D], bf16, tag="wproj_k")
                    nc.sync.dma_start(wk_sb, wk_v)
                    xb = p1.tile([128, DC, L], bf16, tag="xb")
                    nc.sync.dma_start(xb, xb_v)
                    ident = consts.tile([128, 128], bf16, tag="ident")
                    nc.sync.dma_start(ident, ident_d)

                    # constants (small DMAs, off the critical path)
                    ones_bf = consts.tile([128, 1], bf16, tag="ones")
                    nc.vector.memset(ones_bf, 1.0)
                    eps_t = consts.tile([1, 1], f32, tag="eps")
                    nc.vector.memset(eps_t, EPS)
                    bb1_sb = consts.tile([128, FC], f32, tag="bb1")
                    nc.sync.dma_start(bb1_sb, bb1_v)
                    bb2_sb = consts.tile([128, DC], f32, tag="bb2")
                    nc.sync.dma_start(bb2_sb, bb2_v)
                    g1_sb = consts.tile([128, DC], f32, tag="g1")
                    nc.sync.dma_start(g1_sb, g1_v)
                    b1_sb = consts.tile([128, DC], f32, tag="b1")
                    nc.sync.dma_start(b1_sb, b1_v)
                    g2_sb = consts.tile([128, DC], f32, tag="g2")
                    nc.sync.dma_start(g2_sb, g2_v)
                    b2_sb = consts.tile([128, DC], f32, tag="b2")
                    nc.sync.dma_start(b2_sb, b2_v)
                    nc.vector.memset(kaug_h[:, :, :, 64:65], 1.0)

                    # qT = (x_q @ wq)^T  (1/sqrt(HD) folded into wq on the host)
                    for co in range(DC):
                        ps = psA.tile([128, 512], f32, tag="ps")
                        for c in range(DC):
                            nc.tensor.matmul(ps, wq_sb[:, c, co * 128:(co + 1) * 128],
                                             xqb[:, c, :], start=(c == 0), stop=(c == DC - 1))
                        nc.vector.tensor_copy(qT[:, co, :], ps)

                    # kT = (x @ wk)^T  over the full sequence
                    for co in range(DC):
                        for mt in range(L // 512):
                            ps = psA.tile([128, 512], f32, tag="ps")
                            for c in range(DC):
                                nc.tensor.matmul(ps, wk_sb[:, c, co * 128:(co + 1) * 128],
                                                 xb[:, c, mt * 512:(mt + 1) * 512],
                                                 start=(c == 0), stop=(c == DC - 1))
                            nc.vector.tensor_copy(kT[:, co, mt * 512:(mt + 1) * 512], ps)

                    # kaug = kT^T via PE transposes (4 feature-chunks per PSUM tile)
                    with tc.tile_pool(name="psT", bufs=4, space="PSUM") as psT:
                        for mi in range(MC):
                            for g in range(2):
                                pt = psT.tile([128, 512], bf16, tag="pt")
                                for j in range(4):
                                    c = g * 4 + j
                                    nc.tensor.transpose(
                                        pt[:, j * 128:(j + 1) * 128],
                                        kT[:, c, mi * 128:(mi + 1) * 128], ident)
                                nc.vector.tensor_copy(
                                    kaug_h[:, mi, g * 8:(g + 1) * 8, 0:64],
                                    pt.rearrange("p (h e) -> p h e", e=64))

                # ---- Phase 2: attention ----
                with tc.tile_pool(name="attn", bufs=1) as attn, \
                     tc.tile_pool(name="epool", bufs=2) as epool, \
                     tc.tile_pool(name="wop", bufs=1) as wop, \
                     tc.tile_pool(name="psS", bufs=2, space="PSUM") as psS, \
                     tc.tile_pool(name="psU", bufs=2, space="PSUM") as psU:
                    # prefetch wo while attention runs
                    wo_sb = wop.tile([128, DC, D], bf16, tag="wo_sb")
                    nc.sync.dma_start(wo_sb, wo_v)

                    ctxT = attn.tile([128, DC, LQ], bf16, tag="ctxT")
                    dens = attn.tile([NH, LQ], f32, tag="dens")

                    for h in range(NH):
                        pair, poff = h // 2, 64 * (h % 2)
                        e = epool.tile([128, MC, LQ], bf16, tag="E")
                        for mt in range(4):
                            st = psS.tile([128, 1024], f32, tag="st")
                            for j in range(2):
                                mi = mt * 2 + j
                                nc.tensor.matmul(
                                    st[:, j * 512:(j + 1) * 512],
                                    kT[poff:poff + 64, pair, mi * 128:(mi + 1) * 128],
                                    qT[poff:poff + 64, pair, :],
                                    start=True, stop=True)
                            nc.scalar.activation(
                                e[:, mt * 2:(mt + 1) * 2, :].rearrange("p a b -> p (a b)"),
                                st, AF.Exp)
                        u = psU.tile([128, 512], f32, tag="u")
                        for mi in range(MC):
                            nc.tensor.matmul(u[0:65, :],
                                             kaug[:, mi, h * 65:h * 65 + 65],
                                             e[:, mi, :],
                                             start=(mi == 0), stop=(mi == MC - 1))
                        nc.vector.tensor_copy(ctxT[poff:poff + 64, pair, :], u[0:64, :])
                        drow = sm2.tile([1, LQ], f32, tag="drow")
                        nc.vector.tensor_copy(drow, u[64:65, :])
                        nc.sync.dma_start(dens[h:h + 1, :], drow)

                    # batched softmax denominators: one reciprocal, one bcast, one mult
                    rec32 = sm.tile([NH, LQ], f32, tag="rec32")
                    nc.vector.reciprocal(rec32, dens)
                    rec16 = sm.tile([NH, LQ], bf16, tag="rec16")
                    nc.vector.tensor_copy(rec16, rec32)
                    den_bc = attn.tile([128, DC, LQ], bf16, tag="den_bc")
                    sc = dramsc.tile([NH, LQ], bf16, tag="rec_sc", name="rec_sc")
                    nc.sync.dma_start(sc, rec16)
                    for h in range(NH):
                        pair, poff = h // 2, 64 * (h % 2)
                        nc.sync.dma_start(den_bc[poff:poff + 64, pair, :],
                                          sc[h].partition_broadcast(64))
                    nc.vector.tensor_tensor(
                        ctxT.rearrange("p c l -> p (c l)"),
                        ctxT.rearrange("p c l -> p (c l)"),
                        den_bc.rearrange("p c l -> p (c l)"), OP.mult)

                    # attn_out + residual -> r1T  (xq streamed per f-chunk)
                    with tc.tile_pool(name="psB", bufs=2, space="PSUM") as psB:
                        for f in range(DC):
                            xq_t = sm2.tile([128, 512], f32, tag="xq_t")
                            nc.sync.dma_start(xq_t, xq_v[:, f, :])
                            ps = psB.tile([128, 512], f32, tag="ao")
                            for c in range(DC):
                                nc.tensor.matmul(ps, wo_sb[:, c, f * 128:(f + 1) * 128],
                                                 ctxT[:, c, :], start=(c == 0), stop=(c == DC - 1))
                            nc.vector.tensor_tensor(r1T[:, f, :], ps, xq_t, OP.add)

                # ---- LN1 ----  (r1T -> hT f32 + hb bf16)
                def layer_norm_T(rT, gam_sb, bet_sb, outT, psL, tmp_pool, out_bf=None,
                                 out_pool=None, out_dram=None):
                    sq = tmp_pool.tile([128, DC, LQ], bf16, tag="ln_sq")
                    if rT.dtype == f32:
                        rb = tmp_pool.tile([128, DC, LQ], bf16, tag="ln_rb")
                        for c in range(DC):
                            nc.vector.tensor_copy(rb[:, c, :], rT[:, c, :])
                    else:
                        rb = rT
                    for c in range(DC):
                        nc.scalar.activation(sq[:, c, :], rT[:, c, :], AF.Square)
                    s_ps = psL.tile([1, LQ], f32, tag="ln_sum_r")
                    q_ps = psL.tile([1, LQ], f32, tag="ln_sum_s")
                    for c in range(DC):
                        nc.tensor.matmul(s_ps, ones_bf, rb[:, c, :],
                                         start=(c == 0), stop=(c == DC - 1))
                    for c in range(DC):
                        nc.tensor.matmul(q_ps, ones_bf, sq[:, c, :],
                                         start=(c == 0), stop=(c == DC - 1))
                    mu = sm.tile([1, LQ], f32, tag="ln_mu")
                    nc.scalar.activation(mu, s_ps, AF.Copy, scale=1.0 / D)
                    msq = sm.tile([1, LQ], f32, tag="ln_msq")
                    nc.scalar.activation(msq, q_ps, AF.Copy, scale=1.0 / D)
                    var = sm.tile([1, LQ], f32, tag="ln_var")
                    nc.vector.tensor_tensor(var, mu, mu, OP.mult)
                    nc.vector.tensor_tensor(var, msq, var, OP.subtract)
                    std = sm.tile([1, LQ], f32, tag="ln_std")
                    nc.scalar.activation(std, var, AF.Sqrt, bias=eps_t)
                    rstd = sm.tile([1, LQ], f32, tag="ln_rstd")
                    nc.vector.reciprocal(rstd, std)
                    mu_bc = tmp_pool.tile([128, LQ], f32, tag="ln_mubc")
                    bcast(mu_bc, mu, "ln_mu")
                    rstd_bc = tmp_pool.tile([128, LQ], f32, tag="ln_rstdbc")
                    bcast(rstd_bc, rstd, "ln_rstd")
                    cen = tmp_pool.tile([128, LQ], f32, tag="ln_cen")
                    for c in range(DC):
                        nc.vector.tensor_tensor(cen, rT[:, c, :], mu_bc, OP.subtract)
                        nc.vector.tensor_tensor(cen, cen, rstd_bc, OP.mult)
                        if out_dram is not None:
                            oc = out_pool.tile([128, LQ], f32, tag="ln_oc")
                            nc.scalar.activation(oc, cen, AF.Identity,
                                                 scale=gam_sb[:, c:c + 1], bias=bet_sb[:, c:c + 1])
                            nc.sync.dma_start(out_dram[:, c, :], oc)
                        else:
                            nc.scalar.activation(outT[:, c, :], cen, AF.Identity,
                                                 scale=gam_sb[:, c:c + 1], bias=bet_sb[:, c:c + 1])
                            if out_bf is not None:
                                nc.vector.tensor_copy(out_bf[:, c, :], outT[:, c, :])

                with tc.tile_pool(name="ln1tmp", bufs=1) as ln1tmp, \
                     tc.tile_pool(name="psL1", bufs=1, space="PSUM") as psL1:
                    layer_norm_T(r1T, g1_sb, b1_sb, hT, psL1, ln1tmp, out_bf=hb)

            # ---- Phase 3: FFN ----
            with tc.tile_pool(name="ffn", bufs=1) as ffn, \
                 tc.tile_pool(name="w1stream", bufs=4) as w1stream, \
                 tc.tile_pool(name="w2pool", bufs=1) as w2pool, \
                 tc.tile_pool(name="psZ", bufs=2, space="PSUM") as psZ, \
                 tc.tile_pool(name="psO", bufs=1, space="PSUM") as psO:
                g_sb = ffn.tile([128, FC, LQ], bf16, tag="g")
                r2T = ffn.tile([128, DC, LQ], f32, tag="r2T")
                w2_sb = w2pool.tile([128, FC, D], bf16, tag="w2_sb")
                for i in range(FC):
                    nc.sync.dma_start(w2_sb[:, i, :], w2_d[i])

                for half in range(2):
                    o_ps = [psO.tile([128, 512], f32, tag=f"o{f}", name=f"o_ps{f}")
                            for f in range(4)]
                    for i in range(FC):
                        if half == 0:
                            w1t = w1stream.tile([128, D], bf16, tag="w1t")
                            nc.sync.dma_start(w1t, w1_d[i])
                            zt = psZ.tile([128, 512], f32, tag="zt")
                            for c in range(DC):
                                nc.tensor.matmul(zt, w1t[:, c * 128:(c + 1) * 128],
                                                 hb[:, c, :], start=(c == 0), stop=(c == DC - 1))
                            nc.scalar.activation(g_sb[:, i, :], zt, AF.Gelu,
                                                 bias=bb1_sb[:, i:i + 1])
                        for f in range(4):
                            nc.tensor.matmul(o_ps[f],
                                             w2_sb[:, i, half * 512 + f * 128:half * 512 + (f + 1) * 128],
                                             g_sb[:, i, :], start=(i == 0), stop=(i == FC - 1))
                    for f in range(4):
                        fo = half * 4 + f
                        t = sm2.tile([128, 512], f32, tag="obias")
                        nc.scalar.activation(t, o_ps[f], AF.Identity, bias=bb2_sb[:, fo:fo + 1])
                        nc.vector.tensor_tensor(r2T[:, fo, :], t, hT[:, fo, :], OP.add)

                # ---- LN2 -> out (chunked DMA) ----
                with tc.tile_pool(name="ln2tmp", bufs=1) as ln2tmp, \
                     tc.tile_pool(name="ln2out", bufs=3) as ln2out, \
                     tc.tile_pool(name="psL2", bufs=1, space="PSUM") as psL2:
                    layer_norm_T(r2T, g2_sb, b2_sb, None, psL2, ln2tmp,
                                 out_pool=ln2out, out_dram=out_v)

    nc.compile()
    return nc


def _get_nc():
    if "nc" not in _cache:
        _cache["nc"] = _build_nc()
    return _cache["nc"]


def _host_prep(inputs):
    x = np.asarray(inputs["x"], np.float32)
    wq = np.asarray(inputs["wq"], np.float32)
    wk = np.asarray(inputs["wk"], np.float32)
    wo = np.asarray(inputs["wo"], np.float32)
    g1 = np.asarray(inputs["g1"], np.float32)
    b1 = np.asarray(inputs["b1"], np.float32)
    w1 = np.asarray(inputs["w1"], np.float32)
    bb1 = np.asarray(inputs["bb1"], np.float32)
    w2 = np.asarray(inputs["w2"], np.float32)
    bb2 = np.asarray(inputs["bb2"], np.float32)
    g2 = np.asarray(inputs["g2"], np.float32)
    b2 = np.asarray(inputs["b2"], np.float32)

    idx = np.arange(D)
    perm = (idx % HD) * NH + (idx // HD)  # f' = h*64+d  ->  old f = d*16+h

    def bf(a):
        return np.ascontiguousarray(a).astype(BF16NP)

    w1t = w1.reshape(DC, 128, FC, 128).transpose(2, 1, 0, 3).reshape(FC, 128, D)
    w2t = w2.reshape(FC, 128, D)
    shared = {
        "wq": bf(wq[:, perm] * (1.0 / np.sqrt(HD))), "wk": bf(wk[:, perm]),
        "wo": bf(wo[perm, :]),
        "w1": bf(w1t), "w2": bf(w2t),
        "ident": bf(np.eye(128, dtype=np.float32)),
        "bb1": bb1, "bb2": bb2, "g1": g1, "b1": b1, "g2": g2, "b2": b2,
    }
    in_maps = []
    for c in range(NCORES):
        b, q0 = c // (NCORES // B), (c % (NCORES // B)) * LQ
        xT = np.ascontiguousarray(x[b].T)
        m = dict(shared)
        m["xb"] = bf(xT)
        m["xqb"] = bf(xT[:, q0:q0 + LQ])
        m["xq"] = np.ascontiguousarray(xT[:, q0:q0 + LQ])
        in_maps.append(m)
    return in_maps


def kernel(**inputs):
    global LAST_RESULTS
    from concourse.bass_utils import run_bass_kernel_spmd

    nc = _get_nc()
    in_maps = _host_prep(inputs)
    res = run_bass_kernel_spmd(nc, in_maps, core_ids=list(range(NCORES)))
    LAST_RESULTS = res
    out = np.empty((B, L, D), np.float32)
    for c in range(NCORES):
        b, q0 = c // (NCORES // B), (c % (NCORES // B)) * LQ
        out[b, q0:q0 + LQ, :] = res.results[c]["out"].T
    return out
